# revision 11
# baseline (speedup 1.0000x reference)
"""Distributed Trainium2 Bass kernel for L2-distance attention.

Math (per batch b):
    q = x @ Wq.T ; k = x @ Wk.T ; v = x @ Wv.T          [N, D]
    att = softmax(sqrt(||q_i - k_j||^2) / sqrt(D), axis=j)
    out = att @ v

Distribution: pure data-parallel over batch. 16 batches / 8 cores = 2
batches per core, weights replicated, no collectives.

Per-core algorithm (all matmuls bf16, fp32 PSUM accumulate):
  - xT [D, N] built via bf16 DRAM round trip + XBAR DMA transpose.
  - qT, kT kept transposed [D, N]; v kept natural [N, D].
  - Scores are computed TRANSPOSED in [128 j, 1024 i] PSUM tiles:
    acc[j, i] = q_i . k_j - q2_i/2 (q2 folded in as a rank-1 matmul).
    ACT pass 1 (Sqrt set): sqrt(-acc/256 + k2_j/512) = dist/sqrt(D),
    with k2/512 as the per-partition bias -> bf16 logit tile.
    ACT pass 2 (Exp set): exp in place -> pT bf16.
    Passes are phase-batched per i-chunk so the ACT table set switches
    only twice per chunk instead of per-tile (a table load is ~2.7us).
  - out rows: psum_o[i, :] = sum_j pT[j, i] * v[j, :]  (pT is already
    the needed lhsT layout - no attention transpose needed); softmax
    denominator comes from a ones-column matmul over pT, transposed to
    a per-partition column via a tiny rank-1 matmul.
"""

import sys

if "/opt/trn_rl_repo" not in sys.path:
    sys.path.insert(0, "/opt/trn_rl_repo")

import numpy as np

B, N, D = 16, 2048, 512
NCORES = 8
BL = B // NCORES  # batches per core
P = 128
ND = D // P       # 4   d-chunks of 128
NB = N // P       # 16  n-chunks of 128
NI = N // 512     # 4   n-chunks of 512
IW = 1024         # i-chunk width for the scores/av phase
NIW = N // IW     # 2

_cache = {}


def _build():
    import concourse.bass as bass
    import concourse.tile as tile
    from concourse.tile import add_dep_helper
    from concourse import bacc, mybir, masks
    from contextlib import ExitStack

    ts = bass.ts
    f32, bf16 = mybir.dt.float32, mybir.dt.bfloat16
    f8 = mybir.dt.float8e4
    DR = mybir.MatmulPerfMode.DoubleRow
    AF = mybir.ActivationFunctionType

    nc = bacc.Bacc("TRN2", target_bir_lowering=False, debug=False, num_devices=NCORES)
    x_ap = nc.dram_tensor("x", [BL, N, D], f32, kind="ExternalInput").ap()
    wq_ap = nc.dram_tensor("Wq", [D, D], f32, kind="ExternalInput").ap()
    wk_ap = nc.dram_tensor("Wk", [D, D], f32, kind="ExternalInput").ap()
    wv_ap = nc.dram_tensor("Wv", [D, D], f32, kind="ExternalInput").ap()
    out_ap = nc.dram_tensor("out", [BL, N, D], f32, kind="ExternalOutput").ap()

    with tile.TileContext(nc) as tc, ExitStack() as ctx:
        pool = lambda **kw: ctx.enter_context(tc.tile_pool(**kw))
        const = pool(name="const", bufs=1)
        stagef = pool(name="stagef", bufs=4)
        stageb = pool(name="stageb", bufs=8)
        wpool = pool(name="wpool", bufs=1)
        xTp = pool(name="xTp", bufs=2)
        qk8p = pool(name="qk8p", bufs=1)
        vp = pool(name="vp", bufs=20)
        pTp = pool(name="pTp", bufs=32)
        sqp = pool(name="sqp", bufs=4)
        outp = pool(name="outp", bufs=3)
        rows = pool(name="rows", bufs=1)
        cols = pool(name="cols", bufs=2)
        ps_s = pool(name="ps_s", bufs=2, space="PSUM")
        ps_cmn = pool(name="ps_cmn", bufs=2, space="PSUM")  # proj + av epilogues
        ps_sm = pool(name="ps_sm", bufs=2, space="PSUM")
        ps_tr = pool(name="ps_tr", bufs=2, space="PSUM")    # PE-transpose staging

        ones_row = const.tile([1, P], bf16)   # lhsT for the q2 rank-1 fold
        nc.gpsimd.memset(ones_row[:], 1.0)
        ones_col = const.tile([P, 1], bf16)   # lhsT for partition-sum rows
        nc.gpsimd.memset(ones_col[:], 1.0)
        mhalf_col = const.tile([P, 1], bf16)  # -0.5: q2 row comes out pre-scaled
        nc.gpsimd.memset(mhalf_col[:], -0.5)
        one_f32 = const.tile([1, 1], f32)     # rhs for [1,128]->[128,1] transposes
        nc.gpsimd.memset(one_f32[:], 1.0)
        one_bf = const.tile([1, 1], bf16)
        nc.gpsimd.memset(one_bf[:], 1.0)
        ident = const.tile([P, P], bf16)
        masks.make_identity(nc, ident[:])

        # ---- weights: f32 -> bf16 -> PE-transpose -> WT[d, o] ----
        WT = {}
        for wname, w_ap in (("q", wq_ap), ("k", wk_ap), ("v", wv_ap)):
            tbs = []
            for r in range(ND):
                tf = stagef.tile([P, D], f32, tag="stagef")
                nc.sync.dma_start(tf[:], w_ap[ts(r, P), :])
                tb = stageb.tile([P, D], bf16, tag="stageb")
                nc.vector.tensor_copy(tb[:], tf[:])
                tbs.append(tb)
            chunks = []
            for dc in range(ND):
                wt = wpool.tile([P, D], bf16, tag=f"wt_{wname}{dc}")
                for r in range(ND):
                    pstr = ps_tr.tile([P, P], bf16, tag="tr")
                    nc.tensor.transpose(pstr[:], tbs[r][:, ts(dc, P)], ident[:])
                    nc.vector.tensor_copy(wt[:, ts(r, P)], pstr[:])
                chunks.append(wt)
            WT[wname] = chunks

        last_exp = None  # ACT-phase ordering: stop Sqrt/Exp table-set thrash
        for b in range(BL):
            # ---- xT via f32 load -> bf16 convert -> PE transpose,
            # quarter-granular so the first projections start early ----
            xT = []
            for dc in range(ND):
                xt_tile = xTp.tile([P, N], bf16, tag=f"xT{dc}")
                xT.append(xt_tile)
            for q in range(4):
                tbs = []
                for r in range(4):
                    tf = stagef.tile([P, D], f32, tag="stagef")
                    nc.sync.dma_start(tf[:], x_ap[b][ts(q * 4 + r, P), :])
                    tb = stageb.tile([P, D], bf16, tag="stageb")
                    nc.vector.tensor_copy(tb[:], tf[:])
                    tbs.append(tb)
                for dc in range(ND):
                    for r in range(4):
                        pstr = ps_tr.tile([P, P], bf16, tag="tr")
                        nc.tensor.transpose(pstr[:], tbs[r][:, ts(dc, P)], ident[:])
                        nc.vector.tensor_copy(xT[dc][:, ts(q * 4 + r, P)], pstr[:])

            # ---- projections (q/k as fp8 [128, ND, N] for DoubleRow scores) ----
            qT8 = qk8p.tile([P, ND, N], f8, tag="qT8")
            kT8 = qk8p.tile([P, ND, N], f8, tag="kT8")
            for dst, wn in ((qT8, "q"), (kT8, "k")):
                for oc in range(ND):
                    for i5 in range(NI):
                        ps = ps_cmn.tile([P, 512], f32, tag="cmn")
                        for dc in range(ND):
                            nc.tensor.matmul(
                                ps[:],
                                WT[wn][dc][:, ts(oc, P)],
                                xT[dc][:, ts(i5, 512)],
                                start=(dc == 0),
                                stop=(dc == ND - 1),
                            )
                        nc.vector.tensor_copy(dst[:, oc, ts(i5, 512)], ps[:])
            v = []
            for nb in range(NB):
                ps = ps_cmn.tile([P, 512], f32, tag="cmn")
                for dc in range(ND):
                    nc.tensor.matmul(
                        ps[:],
                        xT[dc][:, ts(nb, P)],
                        WT["v"][dc][:],
                        start=(dc == 0),
                        stop=(dc == ND - 1),
                    )
                vt = vp.tile([P, 512], bf16, tag="v")
                nc.vector.tensor_copy(vt[:], ps[:])
                v.append(vt)

            # ---- q2 (row, scaled by -1/2, bf16) and k2 (cols, scaled 1/512) ----
            q2row = rows.tile([1, N], bf16, tag="q2row")
            k2row = rows.tile([1, N], bf16, tag="k2row")
            for i5 in range(NI):
                psq = ps_sm.tile([1, 512], f32, tag="sm")
                psk = ps_sm.tile([1, 512], f32, tag="sm")
                for oc in range(ND):
                    sq = sqp.tile([P, 512], bf16, tag="sq")
                    nc.vector.tensor_mul(sq[:], qT8[:, oc, ts(i5, 512)], qT8[:, oc, ts(i5, 512)])
                    nc.tensor.matmul(psq[:], mhalf_col[:], sq[:], start=(oc == 0), stop=(oc == ND - 1))
                    sk = sqp.tile([P, 512], bf16, tag="sq")
                    nc.vector.tensor_mul(sk[:], kT8[:, oc, ts(i5, 512)], kT8[:, oc, ts(i5, 512)])
                    nc.tensor.matmul(psk[:], ones_col[:], sk[:], start=(oc == 0), stop=(oc == ND - 1))
                nc.vector.tensor_copy(q2row[0:1, ts(i5, 512)], psq[:])
                nc.vector.tensor_scalar_mul(k2row[0:1, ts(i5, 512)], psk[:], 1.0 / 512.0)
            k2cols = []
            for jc in range(NB):
                pst = ps_sm.tile([P, 1], f32, tag="sm")
                nc.tensor.matmul(pst[:], k2row[0:1, ts(jc, P)], one_bf[:], start=True, stop=True)
                kc = cols.tile([P, 1], f32, tag=f"k2col{jc}")
                nc.vector.tensor_copy(kc[:], pst[:])
                k2cols.append(kc)

            # ---- scores (transposed) -> logits -> exp -> A @ V ----
            for ibig in range(NIW):
                # phase 1: scores matmuls + Sqrt (one table set for the phase)
                pTs = []
                for jc in range(NB):
                    pt = pTp.tile([P, IW], bf16, tag="pT")
                    for h in range(IW // 512):
                        i5 = ibig * (IW // 512) + h
                        ps = ps_s.tile([P, 512], f32, tag="s")
                        for t in range(0, ND, 2):
                            nc.tensor.matmul(
                                ps[:],
                                kT8[:, t : t + 2, ts(jc, P)],
                                qT8[:, t : t + 2, ts(i5, 512)],
                                start=(t == 0),
                                stop=False,
                                perf_mode=DR,
                            )
                        nc.tensor.matmul(
                            ps[:],
                            ones_row[:],
                            q2row[0:1, ts(i5, 512)],
                            start=False,
                            stop=True,
                        )
                        # logits: sqrt(-acc/256 + k2_j/512) = dist/sqrt(D)
                        sq_i = nc.scalar.activation(
                            pt[:, ts(h, 512)], ps[:], AF.Sqrt,
                            bias=k2cols[jc][:], scale=-1.0 / 256.0,
                        )
                        if last_exp is not None:
                            add_dep_helper(sq_i.ins, last_exp.ins, sync=False,
                                           reason="ACT table-set phase order")
                        last_sqrt = sq_i
                    pTs.append(pt)
                # phase 2: exp in place (single Exp-set load per chunk)
                for jc in range(NB):
                    ex_i = nc.scalar.activation(pTs[jc][:], pTs[jc][:], AF.Exp)
                    add_dep_helper(ex_i.ins, last_sqrt.ins, sync=False,
                                   reason="ACT table-set phase order")
                    last_exp = ex_i
                # softmax denominator row for this i-chunk
                denrow = rows.tile([1, IW], f32, tag="denrow")
                for h in range(IW // 512):
                    psd = ps_sm.tile([1, 512], f32, tag="sm")
                    for jc in range(NB):
                        nc.tensor.matmul(
                            psd[:], ones_col[:], pTs[jc][:, ts(h, 512)],
                            start=(jc == 0), stop=(jc == NB - 1),
                        )
                    nc.vector.tensor_copy(denrow[0:1, ts(h, 512)], psd[:])
                # A @ V rows + normalize
                for isub in range(IW // P):
                    pso = ps_cmn.tile([P, 512], f32, tag="cmn")
                    for jc in range(NB):
                        nc.tensor.matmul(
                            pso[:],
                            pTs[jc][:, ts(isub, P)],
                            v[jc][:],
                            start=(jc == 0),
                            stop=(jc == NB - 1),
                        )
                    psdc = ps_sm.tile([P, 1], f32, tag="sm")
                    nc.tensor.matmul(psdc[:], denrow[0:1, ts(isub, P)], one_f32[:], start=True, stop=True)
                    rcp = cols.tile([P, 1], f32, tag=f"rcp{isub}")
                    nc.vector.reciprocal(rcp[:], psdc[:])
                    ot = outp.tile([P, 512], f32, tag="outs")
                    nc.vector.tensor_scalar_mul(ot[:], pso[:], rcp[:])
                    nc.sync.dma_start(out_ap[b][ts(ibig * (IW // P) + isub, P), :], ot[:])

    nc.compile()
    return nc


def kernel(**inputs):
    from concourse.bass_utils import run_bass_kernel_spmd

    nc = _cache.get("nc")
    if nc is None:
        nc = _cache["nc"] = _build()

    x = np.ascontiguousarray(inputs["x"], dtype=np.float32)
    wq = np.ascontiguousarray(inputs["Wq"], dtype=np.float32)
    wk = np.ascontiguousarray(inputs["Wk"], dtype=np.float32)
    wv = np.ascontiguousarray(inputs["Wv"], dtype=np.float32)

    in_maps = [
        {"x": np.ascontiguousarray(x[c * BL : (c + 1) * BL]), "Wq": wq, "Wk": wk, "Wv": wv}
        for c in range(NCORES)
    ]
    res = run_bass_kernel_spmd(nc, in_maps, core_ids=list(range(NCORES)))
    return np.concatenate([res.results[c]["out"] for c in range(NCORES)], axis=0)


# revision 12
# speedup vs baseline: 1.0087x; 1.0087x over previous
"""Distributed Trainium2 Bass kernel for L2-distance attention.

Math (per batch b):
    q = x @ Wq.T ; k = x @ Wk.T ; v = x @ Wv.T          [N, D]
    att = softmax(sqrt(||q_i - k_j||^2) / sqrt(D), axis=j)
    out = att @ v

Distribution: pure data-parallel over batch. 16 batches / 8 cores = 2
batches per core, weights replicated, no collectives.

Per-core algorithm (all matmuls bf16, fp32 PSUM accumulate):
  - xT [D, N] built via bf16 DRAM round trip + XBAR DMA transpose.
  - qT, kT kept transposed [D, N]; v kept natural [N, D].
  - Scores are computed TRANSPOSED in [128 j, 1024 i] PSUM tiles:
    acc[j, i] = q_i . k_j - q2_i/2 (q2 folded in as a rank-1 matmul).
    ACT pass 1 (Sqrt set): sqrt(-acc/256 + k2_j/512) = dist/sqrt(D),
    with k2/512 as the per-partition bias -> bf16 logit tile.
    ACT pass 2 (Exp set): exp in place -> pT bf16.
    Passes are phase-batched per i-chunk so the ACT table set switches
    only twice per chunk instead of per-tile (a table load is ~2.7us).
  - out rows: psum_o[i, :] = sum_j pT[j, i] * v[j, :]  (pT is already
    the needed lhsT layout - no attention transpose needed); softmax
    denominator comes from a ones-column matmul over pT, transposed to
    a per-partition column via a tiny rank-1 matmul.
"""

import sys

if "/opt/trn_rl_repo" not in sys.path:
    sys.path.insert(0, "/opt/trn_rl_repo")

import numpy as np

B, N, D = 16, 2048, 512
NCORES = 8
BL = B // NCORES  # batches per core
P = 128
ND = D // P       # 4   d-chunks of 128
NB = N // P       # 16  n-chunks of 128
NI = N // 512     # 4   n-chunks of 512
IW = 1024         # i-chunk width for the scores/av phase
NIW = N // IW     # 2

_cache = {}


def _build():
    import concourse.bass as bass
    import concourse.tile as tile
    from concourse.tile import add_dep_helper
    from concourse import bacc, mybir, masks
    from contextlib import ExitStack

    ts = bass.ts
    f32, bf16 = mybir.dt.float32, mybir.dt.bfloat16
    f8 = mybir.dt.float8e4
    DR = mybir.MatmulPerfMode.DoubleRow
    AF = mybir.ActivationFunctionType

    nc = bacc.Bacc("TRN2", target_bir_lowering=False, debug=False, num_devices=NCORES)
    x_ap = nc.dram_tensor("x", [BL, N, D], f32, kind="ExternalInput").ap()
    wq_ap = nc.dram_tensor("Wq", [D, D], f32, kind="ExternalInput").ap()
    wk_ap = nc.dram_tensor("Wk", [D, D], f32, kind="ExternalInput").ap()
    wv_ap = nc.dram_tensor("Wv", [D, D], f32, kind="ExternalInput").ap()
    out_ap = nc.dram_tensor("out", [BL, N, D], f32, kind="ExternalOutput").ap()

    with tile.TileContext(nc) as tc, ExitStack() as ctx:
        pool = lambda **kw: ctx.enter_context(tc.tile_pool(**kw))
        const = pool(name="const", bufs=1)
        stagef = pool(name="stagef", bufs=4)
        stageb = pool(name="stageb", bufs=8)
        wpool = pool(name="wpool", bufs=1)
        xTp = pool(name="xTp", bufs=2)
        qkp = pool(name="qkp", bufs=1)
        dram = pool(name="dram", bufs=1, space="DRAM")
        vp = pool(name="vp", bufs=20)
        pTp = pool(name="pTp", bufs=32)
        sqp = pool(name="sqp", bufs=4)
        outp = pool(name="outp", bufs=3)
        rows = pool(name="rows", bufs=1)
        cols = pool(name="cols", bufs=2)
        ps_s = pool(name="ps_s", bufs=2, space="PSUM")
        ps_cmn = pool(name="ps_cmn", bufs=2, space="PSUM")  # proj + av epilogues
        ps_sm = pool(name="ps_sm", bufs=2, space="PSUM")
        ps_tr = pool(name="ps_tr", bufs=2, space="PSUM")    # PE-transpose staging

        ones_row = const.tile([1, P], bf16)   # lhsT for the q2 rank-1 fold
        nc.gpsimd.memset(ones_row[:], 1.0)
        ones_col = const.tile([P, 1], bf16)   # lhsT for partition-sum rows
        nc.gpsimd.memset(ones_col[:], 1.0)
        mhalf_col = const.tile([P, 1], bf16)  # -0.5: q2 row comes out pre-scaled
        nc.gpsimd.memset(mhalf_col[:], -0.5)
        one_f32 = const.tile([1, 1], f32)     # rhs for [1,128]->[128,1] transposes
        nc.gpsimd.memset(one_f32[:], 1.0)
        one_bf = const.tile([1, 1], bf16)
        nc.gpsimd.memset(one_bf[:], 1.0)
        ident = const.tile([P, P], bf16)
        masks.make_identity(nc, ident[:])

        # ---- weights: f32 -> bf16 -> PE-transpose -> WT[d, o] ----
        WT = {}
        for wname, w_ap in (("q", wq_ap), ("k", wk_ap), ("v", wv_ap)):
            tbs = []
            for r in range(ND):
                tf = stagef.tile([P, D], f32, tag="stagef")
                nc.sync.dma_start(tf[:], w_ap[ts(r, P), :])
                tb = stageb.tile([P, D], bf16, tag="stageb")
                nc.vector.tensor_copy(tb[:], tf[:])
                tbs.append(tb)
            chunks = []
            for dc in range(ND):
                wt = wpool.tile([P, D], bf16, tag=f"wt_{wname}{dc}")
                for r in range(ND):
                    pstr = ps_tr.tile([P, P], bf16, tag="tr")
                    nc.tensor.transpose(pstr[:], tbs[r][:, ts(dc, P)], ident[:])
                    nc.vector.tensor_copy(wt[:, ts(r, P)], pstr[:])
                chunks.append(wt)
            WT[wname] = chunks

        last_exp = None  # ACT-phase ordering: stop Sqrt/Exp table-set thrash
        for b in range(BL):
            # ---- xT via f32 load -> bf16 convert -> PE transpose,
            # quarter-granular so the first projections start early ----
            xT = []
            for dc in range(ND):
                xt_tile = xTp.tile([P, N], bf16, tag=f"xT{dc}")
                xT.append(xt_tile)
            if b == 0:
                # PE-transpose path: cheapest startup latency
                for q in range(4):
                    tbs = []
                    for r in range(4):
                        tf = stagef.tile([P, D], f32, tag="stagef")
                        nc.sync.dma_start(tf[:], x_ap[b][ts(q * 4 + r, P), :])
                        tb = stageb.tile([P, D], bf16, tag="stageb")
                        nc.vector.tensor_copy(tb[:], tf[:])
                        tbs.append(tb)
                    for dc in range(ND):
                        for r in range(4):
                            pstr = ps_tr.tile([P, P], bf16, tag="tr")
                            nc.tensor.transpose(pstr[:], tbs[r][:, ts(dc, P)], ident[:])
                            nc.vector.tensor_copy(xT[dc][:, ts(q * 4 + r, P)], pstr[:])
            else:
                # XBAR DMA-transpose path: zero PE cost, hidden under batch-0 compute
                xbf = dram.tile([N, D], bf16, tag="xbf")
                for r in range(NB):
                    tf = stagef.tile([P, D], f32, tag="stagef")
                    nc.sync.dma_start(tf[:], x_ap[b][ts(r, P), :])
                    tb = stageb.tile([P, D], bf16, tag="stageb")
                    nc.vector.tensor_copy(tb[:], tf[:])
                    nc.sync.dma_start(xbf[ts(r, P), :], tb[:])
                for dc in range(ND):
                    nc.sync.dma_start_transpose(xT[dc][:, ts(0, N)], xbf[:, ts(dc, P)])

            # ---- projections ----
            qT, kT = [], []
            for lst, wn in ((qT, "q"), (kT, "k")):
                for oc in range(ND):
                    dst = qkp.tile([P, N], bf16, tag=f"{wn}T{oc}")
                    for i5 in range(NI):
                        ps = ps_cmn.tile([P, 512], f32, tag="cmn")
                        for dc in range(ND):
                            nc.tensor.matmul(
                                ps[:],
                                WT[wn][dc][:, ts(oc, P)],
                                xT[dc][:, ts(i5, 512)],
                                start=(dc == 0),
                                stop=(dc == ND - 1),
                            )
                        nc.vector.tensor_copy(dst[:, ts(i5, 512)], ps[:])
                    lst.append(dst)
            v = []
            for nb in range(NB):
                ps = ps_cmn.tile([P, 512], f32, tag="cmn")
                for dc in range(ND):
                    nc.tensor.matmul(
                        ps[:],
                        xT[dc][:, ts(nb, P)],
                        WT["v"][dc][:],
                        start=(dc == 0),
                        stop=(dc == ND - 1),
                    )
                vt = vp.tile([P, 512], bf16, tag="v")
                nc.vector.tensor_copy(vt[:], ps[:])
                v.append(vt)

            # ---- q2 (row, scaled by -1/2, bf16) and k2 (cols, scaled 1/512) ----
            q2row = rows.tile([1, N], bf16, tag="q2row")
            k2row = rows.tile([1, N], bf16, tag="k2row")
            for i5 in range(NI):
                psq = ps_sm.tile([1, 512], f32, tag="sm")
                psk = ps_sm.tile([1, 512], f32, tag="sm")
                for oc in range(ND):
                    sq = sqp.tile([P, 512], bf16, tag="sq")
                    nc.vector.tensor_mul(sq[:], qT[oc][:, ts(i5, 512)], qT[oc][:, ts(i5, 512)])
                    nc.tensor.matmul(psq[:], mhalf_col[:], sq[:], start=(oc == 0), stop=(oc == ND - 1))
                    sk = sqp.tile([P, 512], bf16, tag="sq")
                    nc.vector.tensor_mul(sk[:], kT[oc][:, ts(i5, 512)], kT[oc][:, ts(i5, 512)])
                    nc.tensor.matmul(psk[:], ones_col[:], sk[:], start=(oc == 0), stop=(oc == ND - 1))
                nc.vector.tensor_copy(q2row[0:1, ts(i5, 512)], psq[:])
                nc.vector.tensor_scalar_mul(k2row[0:1, ts(i5, 512)], psk[:], 1.0 / 512.0)
            k2cols = []
            for jc in range(NB):
                pst = ps_sm.tile([P, 1], f32, tag="sm")
                nc.tensor.matmul(pst[:], k2row[0:1, ts(jc, P)], one_bf[:], start=True, stop=True)
                kc = cols.tile([P, 1], f32, tag=f"k2col{jc}")
                nc.vector.tensor_copy(kc[:], pst[:])
                k2cols.append(kc)

            # ---- scores (transposed) -> logits -> exp -> A @ V ----
            for ibig in range(NIW):
                # phase 1: scores matmuls + Sqrt (one table set for the phase)
                pTs = []
                for jc in range(NB):
                    pt = pTp.tile([P, IW], bf16, tag="pT")
                    for h in range(IW // 512):
                        i5 = ibig * (IW // 512) + h
                        ps = ps_s.tile([P, 512], f32, tag="s")
                        for oc in range(ND):
                            nc.tensor.matmul(
                                ps[:],
                                kT[oc][:, ts(jc, P)],
                                qT[oc][:, ts(i5, 512)],
                                start=(oc == 0),
                                stop=False,
                            )
                        nc.tensor.matmul(
                            ps[:],
                            ones_row[:],
                            q2row[0:1, ts(i5, 512)],
                            start=False,
                            stop=True,
                        )
                        # logits: sqrt(-acc/256 + k2_j/512) = dist/sqrt(D)
                        sq_i = nc.scalar.activation(
                            pt[:, ts(h, 512)], ps[:], AF.Sqrt,
                            bias=k2cols[jc][:], scale=-1.0 / 256.0,
                        )
                        if last_exp is not None:
                            add_dep_helper(sq_i.ins, last_exp.ins, sync=False,
                                           reason="ACT table-set phase order")
                        last_sqrt = sq_i
                    pTs.append(pt)
                # phase 2: exp in place (single Exp-set load per chunk)
                for jc in range(NB):
                    ex_i = nc.scalar.activation(pTs[jc][:], pTs[jc][:], AF.Exp)
                    add_dep_helper(ex_i.ins, last_sqrt.ins, sync=False,
                                   reason="ACT table-set phase order")
                    last_exp = ex_i
                # softmax denominator row for this i-chunk
                denrow = rows.tile([1, IW], f32, tag="denrow")
                for h in range(IW // 512):
                    psd = ps_sm.tile([1, 512], f32, tag="sm")
                    for jc in range(NB):
                        nc.tensor.matmul(
                            psd[:], ones_col[:], pTs[jc][:, ts(h, 512)],
                            start=(jc == 0), stop=(jc == NB - 1),
                        )
                    nc.vector.tensor_copy(denrow[0:1, ts(h, 512)], psd[:])
                # A @ V rows + normalize
                for isub in range(IW // P):
                    pso = ps_cmn.tile([P, 512], f32, tag="cmn")
                    for jc in range(NB):
                        nc.tensor.matmul(
                            pso[:],
                            pTs[jc][:, ts(isub, P)],
                            v[jc][:],
                            start=(jc == 0),
                            stop=(jc == NB - 1),
                        )
                    psdc = ps_sm.tile([P, 1], f32, tag="sm")
                    nc.tensor.matmul(psdc[:], denrow[0:1, ts(isub, P)], one_f32[:], start=True, stop=True)
                    rcp = cols.tile([P, 1], f32, tag=f"rcp{isub}")
                    nc.vector.reciprocal(rcp[:], psdc[:])
                    ot = outp.tile([P, 512], f32, tag="outs")
                    nc.vector.tensor_scalar_mul(ot[:], pso[:], rcp[:])
                    nc.sync.dma_start(out_ap[b][ts(ibig * (IW // P) + isub, P), :], ot[:])

    nc.compile()
    return nc


def kernel(**inputs):
    from concourse.bass_utils import run_bass_kernel_spmd

    nc = _cache.get("nc")
    if nc is None:
        nc = _cache["nc"] = _build()

    x = np.ascontiguousarray(inputs["x"], dtype=np.float32)
    wq = np.ascontiguousarray(inputs["Wq"], dtype=np.float32)
    wk = np.ascontiguousarray(inputs["Wk"], dtype=np.float32)
    wv = np.ascontiguousarray(inputs["Wv"], dtype=np.float32)

    in_maps = [
        {"x": np.ascontiguousarray(x[c * BL : (c + 1) * BL]), "Wq": wq, "Wk": wk, "Wv": wv}
        for c in range(NCORES)
    ]
    res = run_bass_kernel_spmd(nc, in_maps, core_ids=list(range(NCORES)))
    return np.concatenate([res.results[c]["out"] for c in range(NCORES)], axis=0)
